# revision 16
# baseline (speedup 1.0000x reference)
"""DGAT attention head on 8 trn2 NeuronCores — branch-split formulation.

leaky is piecewise linear, so exp(L1*leaky(C*(hz1_i+hz2_j)+D0)) splits into
two rank-1-separable branches gated by s_ij = [t_ij >= 0], t = hz1_i+hz2_j+D0/C:
  p_ij = adj*s*E1_i*F1_j*k1 + adj*(1-s)*E2_i*F2_j*k2
Only U1 = adj*s is an N x N object. Per core (1024 query rows i, 8192 keys j
on partitions in 32 "double groups" of 256 = 2 k-tiles x 128):

  numerator = E1 o (U1 @ G1) + E2 o (adj @ G2 - U1 @ G2),  Gb = Fb o [h|1]

G matrices ship from host in fp8-e4m3 (G1 with an fp8 residual stream for
precision) and all masks stream as fp8, so every matmul runs DoubleRow
(0.5 cyc/row). The N x N exp of the baseline disappears entirely.

Three per-dg mask paths balance DVE/ACT/Pool:
  y: madj=(1-adj)*32 e5m2; U1 = (hz1+thr) is_ge madj   (DVE STT, 1 cyc)
     C-stream moving = madj directly.
  z: mp8 = e4m3((1-adj)*256 - thr); d = hz1-mp8 (Pool sub), U1 = d>=0
     (Pool TS); C-moving = ACT Sign(mp8 + thr - 8) in {-1,+1}.
  x: mp16 = f16((1-adj)*16384 - thr); U1 = hz1 is_ge mp16 (DVE TT, 0.5 cyc),
     fp8 convert on ACT; C-moving = ACT Sign as z.
Tail: PSUM -> SBUF, PE transpose to [i-part, d-free], recombine streams with
host-exact scales/colsum row, normalize, ELU.
"""

import numpy as np
import ml_dtypes

import concourse.bass as bass
import concourse.bacc as bacc
import concourse.mybir as mybir
import concourse.dve_ops as dve_ops
from concourse.dve_spec import Spec, Src0, Src1, C0, C1, One, maxx
from concourse.tile import TileContext
from concourse.bass_utils import run_bass_kernel_spmd

F32 = mybir.dt.float32
F16 = mybir.dt.float16
F8E4 = mybir.dt.float8e4
F8E5 = mybir.dt.float8e5
AF = mybir.ActivationFunctionType
OP = mybir.AluOpType
PM = mybir.MatmulPerfMode
E4NP = ml_dtypes.float8_e4m3
E5NP = ml_dtypes.float8_e5m2

NCORES = 8
SLOPE = 0.2
MY = 32.0       # y-path mask magnitude (e5m2-exact, > max |t|)
MZ = 128.0      # z-path mask magnitude (e4m3-exact-ish)
MX = 256.0      # x-path mask magnitude (f16, exact enough)
TMAX = 8.0      # host rescales t so max |t| <= TMAX

# per-double-group mask path: 32 entries of 'x' | 'y' | 'z'
PATHS = (["x", "y", "x", "z", "x"] * 6 + ["y", "y"])

TRACE = False
LAST_RESULTS = None
LAST_NC = None


def _leaky(z):
    return z if z >= 0.0 else SLOPE * z


def _finish_register(name, spec):
    from concourse.dve_spec import lower
    from concourse.dve_ops import has_src1
    from concourse.dve_uop import DveOpSpec

    op = dve_ops.DveOp(name, spec, subdim=False, uops_sha={})
    dve_ops.OPS.append(op)
    dve_ops.CUSTOM_DVE_SPECS[name] = spec
    dve_ops._SUB_OPCODE_FOR_NAME[name] = (
        dve_ops._CUSTOM_DVE_ROW_BASE + len(dve_ops.OPS) - 1
    )
    assert dve_ops._SUB_OPCODE_FOR_NAME[name] < 0x20
    for ver in ("v3",):
        pinned = DveOpSpec(
            name=name,
            opcode=dve_ops.get_dve_sub_opcode(name),
            uops=lower(spec, ver=ver),
            rd1_en=has_src1(spec),
        ).sha(ver)
        op.uops_sha[ver] = pinned
        dve_ops._COMPILE_CACHE.pop((name, ver), None)
        op.compile(ver)
    return op


def _register_c2_op():
    name = "C2_AXPBY_ANT"
    for op in dve_ops.OPS:
        if op.name == name:
            return op
    spec = Spec(
        body=Src0 * C0 + Src1 * C1,
        reference=lambda in0, in1, s0, s1, imm2: (
            in0 * s0 + in1 * s1
        ).astype(np.float32),
    )
    return _finish_register(name, spec)


def _register_elu_max_op():
    name = "ELU_MAX_ANT"
    for op in dve_ops.OPS:
        if op.name == name:
            return op
    spec = Spec(
        body=maxx(Src0 * C0, Src1 - One),
        reference=lambda in0, in1, s0, s1, imm2: np.maximum(
            in0 * s0, in1 - 1.0
        ).astype(np.float32),
    )
    return _finish_register(name, spec)


def _build(rows, inv_s1, inv_sr, negc, negs2):
    """SPMD Bass program (identical on all cores)."""
    ndg = 32
    ny = sum(1 for p in PATHS if p == "y")
    nz = sum(1 for p in PATHS if p == "z")
    nx = sum(1 for p in PATHS if p == "x")
    nch = rows // 128  # 8 tail chunks

    c2 = _register_c2_op()
    emx = _register_elu_max_op()

    nc = bacc.Bacc("TRN2", target_bir_lowering=False)
    adjy_d = nc.dram_tensor("adjy", [max(ny, 1), 128, 2, rows], F8E5,
                            kind="ExternalInput")
    adjz_d = nc.dram_tensor("adjz", [max(nz, 1), 128, 2, rows], F8E4,
                            kind="ExternalInput")
    adjx_d = nc.dram_tensor("adjx", [max(nx, 1), 128, 2, rows], F16,
                            kind="ExternalInput")
    hz1_d = nc.dram_tensor("hz1", [1, rows], F16, kind="ExternalInput")
    thr_d = nc.dram_tensor("thr", [128, 2 * ndg], F32, kind="ExternalInput")
    ga_d = nc.dram_tensor("ga", [128, 2 * ndg, 128], F8E4, kind="ExternalInput")
    gb_d = nc.dram_tensor("gb", [128, 2 * ndg, 80], F8E4, kind="ExternalInput")
    gc_d = nc.dram_tensor("gc", [128, 2 * ndg, 80], F8E4, kind="ExternalInput")
    e1_d = nc.dram_tensor("e1", [128, nch], F32, kind="ExternalInput")
    e2_d = nc.dram_tensor("e2", [128, nch], F32, kind="ExternalInput")
    cs_d = nc.dram_tensor("cs", [1, nch * 65], F32, kind="ExternalInput")
    y_d = nc.dram_tensor("y", [rows, 64], F32, kind="ExternalOutput")

    with TileContext(nc) as tc:
        with (
            tc.tile_pool(name="consts", bufs=1) as consts,
            tc.tile_pool(name="adjyp", bufs=3) as adjyp,
            tc.tile_pool(name="adjzp", bufs=2) as adjzp,
            tc.tile_pool(name="adjxp", bufs=2) as adjxp,
            tc.tile_pool(name="u8p", bufs=3) as u8p,
            tc.tile_pool(name="u16p", bufs=2) as u16p,
            tc.tile_pool(name="d16p", bufs=2) as d16p,
            tc.tile_pool(name="sg8p", bufs=2) as sg8p,
            tc.tile_pool(name="tailp", bufs=1) as tailp,
        ):
            from concourse.masks import make_identity

            identity = consts.tile([128, 128], F32)
            make_identity(nc, identity)
            zcol = consts.tile([128, 1], F32)
            nc.vector.memset(zcol, 0.0)

            hz1bc = consts.tile([128, rows], F16)
            nc.sync.dma_start(
                out=hz1bc,
                in_=bass.AP(tensor=hz1_d[:, :].tensor, offset=0,
                            ap=[[0, 128], [1, rows]]),
            )
            thr = consts.tile([128, 2 * ndg], F32)
            nc.sync.dma_start(out=thr, in_=thr_d[:, :])
            e1c = consts.tile([128, nch], F32)
            nc.sync.dma_start(out=e1c, in_=e1_d[:, :])
            e2c = consts.tile([128, nch], F32)
            nc.sync.dma_start(out=e2c, in_=e2_d[:, :])
            csbc = consts.tile([128, nch * 65], F32)
            nc.sync.dma_start(
                out=csbc,
                in_=bass.AP(tensor=cs_d[:, :].tensor, offset=0,
                            ap=[[0, 128], [1, nch * 65]]),
            )

            # prefetch the first few adj tiles so DVE/Pool start before the
            # (large) G DMAs complete
            NPRE = 6
            pre = {}
            iy = iz = ix = 0
            for dg in range(min(NPRE, ndg)):
                p = PATHS[dg]
                if p == "y":
                    t = adjyp.tile([128, 2, rows], F8E5)
                    nc.sync.dma_start(out=t, in_=adjy_d[iy])
                    iy += 1
                elif p == "z":
                    t = adjzp.tile([128, 2, rows], F8E4)
                    nc.sync.dma_start(out=t, in_=adjz_d[iz])
                    iz += 1
                else:
                    t = adjxp.tile([128, 2, rows], F16)
                    nc.sync.dma_start(out=t, in_=adjx_d[ix])
                    ix += 1
                pre[dg] = t

            ga = consts.tile([128, 2 * ndg, 128], F8E4)
            gb = consts.tile([128, 2 * ndg, 80], F8E4)
            gc = consts.tile([128, 2 * ndg, 80], F8E4)
            for s0 in range(0, 2 * ndg, 16):
                sl = slice(s0, s0 + 16)
                nc.sync.dma_start(out=ga[:, sl, :], in_=ga_d[:, sl, :])
                nc.sync.dma_start(out=gb[:, sl, :], in_=gb_d[:, sl, :])
                nc.sync.dma_start(out=gc[:, sl, :], in_=gc_d[:, sl, :])

            with tc.tile_pool(name="psacc", bufs=1, space="PSUM") as psacc:
                accA = psacc.tile([128, rows], F32)
                accB = psacc.tile([67, rows], F32)
                accC = psacc.tile([65, rows], F32)

                startedC = [False, False]
                for dg in range(ndg):
                    p = PATHS[dg]
                    g0 = 2 * dg
                    u8t = u8p.tile([128, 2, rows], F8E4)
                    if p == "y":
                        if dg in pre:
                            adjt = pre[dg]
                        else:
                            adjt = adjyp.tile([128, 2, rows], F8E5)
                            nc.sync.dma_start(out=adjt, in_=adjy_d[iy])
                            iy += 1
                        for kt in range(2):
                            nc.vector.scalar_tensor_tensor(
                                out=u8t[:, kt, :],
                                in0=hz1bc,
                                scalar=thr[:, g0 + kt : g0 + kt + 1],
                                in1=adjt[:, kt, :],
                                op0=OP.add,
                                op1=OP.is_ge,
                            )
                    elif p == "z":
                        if dg in pre:
                            adjt = pre[dg]
                        else:
                            adjt = adjzp.tile([128, 2, rows], F8E4)
                            nc.sync.dma_start(out=adjt, in_=adjz_d[iz])
                            iz += 1
                        d16t = d16p.tile([128, 2, rows], F16)
                        for kt in range(2):
                            nc.gpsimd.tensor_sub(
                                d16t[:, kt, :], hz1bc, adjt[:, kt, :]
                            )
                        for kt in range(2):
                            nc.gpsimd.tensor_scalar(
                                u8t[:, kt, :], d16t[:, kt, :], 0.0, 1.0,
                                OP.is_ge, OP.mult,
                            )
                    else:
                        if dg in pre:
                            adjt = pre[dg]
                        else:
                            adjt = adjxp.tile([128, 2, rows], F16)
                            nc.sync.dma_start(out=adjt, in_=adjx_d[ix])
                            ix += 1
                        u16t = u16p.tile([128, 2, rows], F16)
                        for kt in range(2):
                            nc.vector.tensor_tensor(
                                u16t[:, kt, :], hz1bc, adjt[:, kt, :],
                                OP.is_ge,
                            )
                        nc.scalar.activation(u8t, u16t, AF.Copy)

                    start = dg == 0
                    stop = dg == ndg - 1
                    for ci, c0 in enumerate(range(0, rows, 512)):
                        cs_ = slice(c0, c0 + 512)
                        nc.tensor.matmul(
                            accA[:, cs_], ga[:, g0 : g0 + 2, :],
                            u8t[:, :, cs_], start=start, stop=stop,
                            perf_mode=PM.DoubleRow,
                        )
                        nc.tensor.matmul(
                            accB[:, cs_], gb[:, g0 : g0 + 2, 0:67],
                            u8t[:, :, cs_], start=start, stop=stop,
                            perf_mode=PM.DoubleRow,
                        )
                        # C-stream: moving data is the raw mask tile itself
                        if p == "x":
                            for kt in range(2):
                                nc.tensor.matmul(
                                    accC[:, cs_],
                                    gc[:, g0 + kt, 0:65],
                                    adjt[:, kt, cs_],
                                    start=not startedC[ci],
                                    stop=stop and kt == 1,
                                    skip_group_check=True,
                                )
                                startedC[ci] = True
                        else:
                            nc.tensor.matmul(
                                accC[:, cs_], gc[:, g0 : g0 + 2, 0:65],
                                adjt[:, :, cs_], start=not startedC[ci],
                                stop=stop, perf_mode=PM.DoubleRow,
                                skip_group_check=True,
                            )
                            startedC[ci] = True

                sA = tailp.tile([128, rows], F32)
                nc.scalar.copy(sA, accA)
                sB = tailp.tile([67, rows], F32)
                nc.vector.tensor_copy(sB, accB)
                sC = tailp.tile([65, rows], F32)
                nc.scalar.copy(sC, accC)

            with tc.tile_pool(name="pstail", bufs=1, space="PSUM") as pstail:
                tpA = pstail.tile([128, nch, 128], F32)
                tpB = pstail.tile([128, nch, 128], F32)
                tpC = pstail.tile([128, nch, 128], F32)
                for cc in range(nch):
                    ccs = slice(cc * 128, (cc + 1) * 128)
                    nc.tensor.transpose(tpA[:, cc, :], sA[:, ccs], identity)
                    nc.tensor.transpose(
                        tpB[:, cc, 0:67], sB[:, ccs], identity[0:67, 0:67]
                    )
                    nc.tensor.transpose(
                        tpC[:, cc, 0:65], sC[:, ccs], identity[0:65, 0:65]
                    )

                sTA = tailp.tile([128, nch, 128], F32)
                nc.scalar.copy(sTA, tpA)
                sTB = tailp.tile([128, nch, 67], F32)
                nc.scalar.copy(sTB, tpB[:, :, 0:67])
                n1 = tailp.tile([128, nch, 65], F32)
                nc.vector._custom_dve(
                    c2, out=n1[:, :, 0:63], in0=sTA[:, :, 0:63],
                    in1=sTA[:, :, 65:128], s0=inv_s1, s1=inv_sr,
                )
                nc.vector._custom_dve(
                    c2, out=n1[:, :, 63:65], in0=sTA[:, :, 63:65],
                    in1=sTB[:, :, 0:2], s0=inv_s1, s1=inv_sr,
                )
                t2 = tailp.tile([128, nch, 65], F32)
                nc.vector._custom_dve(
                    c2, out=t2, in0=tpC[:, :, 0:65], in1=sTB[:, :, 2:67],
                    s0=negc, s1=negs2,
                )
                t2b = tailp.tile([128, nch, 65], F32)
                nc.vector.tensor_tensor(
                    t2b, t2,
                    csbc.rearrange("p (c d) -> p c d", c=nch), OP.add,
                )
                num = tailp.tile([128, nch, 65], F32)
                for cc in range(nch):
                    nc.vector._custom_dve(
                        c2, out=num[:, cc, :], in0=n1[:, cc, :],
                        in1=t2b[:, cc, :], s0=e1c[:, cc : cc + 1],
                        s1=e2c[:, cc : cc + 1],
                    )
                recip = tailp.tile([128, nch], F32)
                for cc in range(nch):
                    nc.vector.reciprocal(
                        recip[:, cc : cc + 1], num[:, cc, 64:65]
                    )
                for cc in range(nch):
                    vm = tailp.tile([128, 64], F32, name=f"vm{cc}")
                    nc.vector.tensor_scalar(
                        vm, num[:, cc, 0:64], recip[:, cc : cc + 1], 0.0,
                        OP.mult, OP.min,
                    )
                    e2x = tailp.tile([128, 64], F32, name=f"e2x{cc}")
                    nc.scalar.activation(e2x, vm, AF.Exp, bias=zcol[:, 0:1])
                    ysb = tailp.tile([128, 64], F32, name=f"ysb{cc}")
                    nc.vector._custom_dve(
                        emx, out=ysb, in0=num[:, cc, 0:64], in1=e2x,
                        s0=recip[:, cc : cc + 1], s1=0.0, imm2=0.0,
                    )
                    nc.sync.dma_start(
                        out=y_d[cc * 128 : (cc + 1) * 128, :], in_=ysb
                    )
    nc.compile()
    return nc


def _pack_sc(G, headroom=2.0):
    m = float(np.abs(G).max())
    if m == 0.0:
        return 1.0
    return 2.0 ** float(np.floor(np.log2(240.0 / (m * headroom))))


def _e4(x):
    return np.asarray(x, np.float32).astype(E4NP)


def _run(x, adj, w, a, a_coeff, b_coeff, c_coeff, d_coeff):
    global LAST_RESULTS, LAST_NC
    n, din = x.shape
    dout = w.shape[1]
    rows = n // NCORES
    ndg = n // 256
    nch = rows // 128
    assert len(PATHS) == ndg

    A = float(np.asarray(a_coeff).reshape(-1)[0])
    B = float(np.asarray(b_coeff).reshape(-1)[0])
    C = float(np.asarray(c_coeff).reshape(-1)[0])
    D0 = float(np.asarray(d_coeff).reshape(-1)[0])
    L1 = _leaky(A + B)

    x = np.ascontiguousarray(x, dtype=np.float32)
    adj = np.asarray(adj, dtype=np.float32)
    assert ((adj == 0.0) | (adj == 1.0)).all(), "adj must be binary"
    w = np.ascontiguousarray(w, dtype=np.float32)
    a = np.ascontiguousarray(a, dtype=np.float32)

    h = (x @ w).astype(np.float32)
    hz1 = (h @ a[:dout, 0]).astype(np.float32)
    hz2 = (h @ a[dout:, 0]).astype(np.float32)

    kp1, kp2 = L1 * C, SLOPE * L1 * C
    a1s, b1s = float((kp1 * hz1).max()), float((kp1 * hz2).max())
    a2s, b2s = float((kp2 * hz1).max()), float((kp2 * hz2).max())
    gamma = max(a1s + b1s + L1 * D0, a2s + b2s + SLOPE * L1 * D0)
    E1 = np.exp(kp1 * hz1 - a1s) * np.exp(a1s + b1s + L1 * D0 - gamma)
    E2 = np.exp(kp2 * hz1 - a2s) * np.exp(a2s + b2s + SLOPE * L1 * D0 - gamma)
    F1 = np.exp(kp1 * hz2 - b1s)
    F2 = np.exp(kp2 * hz2 - b2s)
    he = np.concatenate([h, np.ones((n, 1), np.float32)], axis=1)

    # branch-indicator encoding: s = [hz1e_i + tcol_j >= 0]
    if C > 0.0:
        hz1e, tcol = hz1, hz2 + D0 / C
    elif C < 0.0:
        hz1e, tcol = -hz1, -(hz2 + D0 / C)
    else:
        hz1e = np.zeros_like(hz1)
        tcol = np.full_like(hz2, 8.0 if D0 >= 0.0 else -8.0)
    tmax = float(np.abs(hz1e).max() + np.abs(tcol).max())
    sig = 2.0 ** float(np.floor(np.log2(TMAX / max(tmax, 1e-30))))
    if C != 0.0:
        hz1e, tcol = hz1e * sig, tcol * sig

    # G matrices + packs
    G1 = F1[:, None] * he
    G2 = F2[:, None] * he
    s1 = _pack_sc(G1)
    G1q = _e4(G1 * s1)
    G1r = (G1 * s1 - G1q.astype(np.float32)) / s1
    sr = _pack_sc(G1r)
    G1rq = _e4(G1r * sr)
    s2 = _pack_sc(G2)
    G2q = _e4(G2 * s2)
    scc = _pack_sc(2.0 * G2) / 2.0

    # C-stream stationaries: moving data is the raw mask tile (values
    # alpha_j for masked, beta_j for unmasked); with W = e4(G2*2scc/(a-b)):
    #   tile @ W = sum_j alpha_j W_j - 2scc * adj @ (G2 + noise)
    # y-path moving is {0, MY}: alpha = MY, beta = 0.
    alpha = np.zeros(n, np.float32)
    beta = np.zeros(n, np.float32)
    for dg in range(ndg):
        js = slice(dg * 256, (dg + 1) * 256)
        if PATHS[dg] == "y":
            alpha[js], beta[js] = MY, 0.0
        elif PATHS[dg] == "z":
            alpha[js] = _e4(MZ - tcol[js]).astype(np.float32)
            beta[js] = _e4(-tcol[js]).astype(np.float32)
        else:
            alpha[js] = (MX - tcol[js]).astype(np.float16).astype(np.float32)
            beta[js] = (-tcol[js]).astype(np.float16).astype(np.float32)
    Wc = _e4(G2 * (2.0 * scc) / (alpha - beta)[:, None])

    # colsum constant (exact over quantized stationaries)
    csC = (alpha[:, None].astype(np.float64)
           * Wc.astype(np.float64)).sum(axis=0)
    csrow = np.tile((csC / (2.0 * scc)).astype(np.float32), nch)[None, :]

    def dev_layout(mat, width):
        # [n, width] -> [128, 2*ndg, width] with j = dg*256 + kt*128 + p
        m = mat.reshape(ndg, 2, 128, width).transpose(2, 0, 1, 3)
        return np.ascontiguousarray(m.reshape(128, 2 * ndg, width))

    ga_h = dev_layout(
        np.concatenate([G1q, G1rq[:, 0:63]], axis=1).view(np.uint8), 128
    ).view(E4NP)
    gb_raw = np.zeros((n, 80), E4NP)
    gb_raw[:, 0:2] = G1rq[:, 63:65]
    gb_raw[:, 2:67] = G2q
    gb_h = dev_layout(gb_raw.view(np.uint8), 80).view(E4NP)
    gc_raw = np.zeros((n, 80), E4NP)
    gc_raw[:, 0:65] = Wc
    gc_h = dev_layout(gc_raw.view(np.uint8), 80).view(E4NP)

    def col_layout(v):
        # [n] -> [128, 2*ndg]
        return np.ascontiguousarray(
            v.reshape(ndg, 2, 128).transpose(2, 0, 1).reshape(128, 2 * ndg)
        ).astype(np.float32)

    thr_h = col_layout(tcol)

    inv_s1 = 1.0 / s1
    inv_sr = 1.0 / sr
    negc = -1.0 / (2.0 * scc)
    negs2 = -1.0 / s2

    nc = _build(rows, inv_s1, inv_sr, negc, negs2)
    LAST_NC = nc

    ny = sum(1 for p in PATHS if p == "y")
    nz = sum(1 for p in PATHS if p == "z")
    nx = sum(1 for p in PATHS if p == "x")

    in_maps = []
    for c in range(NCORES):
        rs = slice(c * rows, (c + 1) * rows)
        adjT = adj[rs, :].T  # [n, rows]
        at4 = adjT.reshape(ndg, 2, 128, rows).transpose(0, 2, 1, 3)
        tc4 = tcol.reshape(ndg, 2, 128).transpose(0, 2, 1)
        ya, za, xa = [], [], []
        for dg in range(ndg):
            adj_t = at4[dg]                      # [128, 2, rows]
            tcol_t = tc4[dg][:, :, None]         # [128, 2, 1]
            if PATHS[dg] == "y":
                ya.append(((1.0 - adj_t) * MY).astype(E5NP))
            elif PATHS[dg] == "z":
                za.append(((1.0 - adj_t) * MZ - tcol_t).astype(E4NP))
            else:
                xa.append(((1.0 - adj_t) * MX - tcol_t).astype(np.float16))
        adjy_h = (np.stack(ya) if ya
                  else np.zeros((1, 128, 2, rows), E5NP))
        adjz_h = (np.stack(za) if za
                  else np.zeros((1, 128, 2, rows), E4NP))
        adjx_h = (np.stack(xa) if xa
                  else np.zeros((1, 128, 2, rows), np.float16))
        e1_h = np.ascontiguousarray(
            E1[rs].reshape(nch, 128).T).astype(np.float32)
        e2_h = np.ascontiguousarray(
            E2[rs].reshape(nch, 128).T).astype(np.float32)
        in_maps.append(
            {
                "adjy": np.ascontiguousarray(adjy_h),
                "adjz": np.ascontiguousarray(adjz_h),
                "adjx": np.ascontiguousarray(adjx_h),
                "hz1": hz1e[rs][None, :].astype(np.float16),
                "thr": thr_h,
                "ga": ga_h,
                "gb": gb_h,
                "gc": gc_h,
                "e1": e1_h,
                "e2": e2_h,
                "cs": csrow,
            }
        )

    res = run_bass_kernel_spmd(
        nc, in_maps, core_ids=list(range(NCORES)), trace=TRACE
    )
    LAST_RESULTS = res
    return np.concatenate([r["y"] for r in res.results], axis=0)


def kernel(x, adj, w, a, a_coeff, b_coeff, c_coeff, d_coeff):
    return _run(x, adj, w, a, a_coeff, b_coeff, c_coeff, d_coeff)
